# revision 4
# baseline (speedup 1.0000x reference)
"""Trainium2 Bass kernel: 21 depthwise Gaussian blurs + channel concat.

Problem: x (8, 3, 512, 512) f32 -> out (8, 66, 512, 512) f32 where
out = concat([x, blur_0(x), ..., blur_20(x)], axis=1) and blur_i is a
depthwise 2D Gaussian conv (reflect padding, kernel sizes 3..21).

Strategy (pure data parallel, 1 image per core across 8 cores):
  The 21 Gaussians have sigma in [0.5, 1.0] -- a very low-rank family.
  Only 6 BASIS blurs {0,5,8,12,16,20} are computed exactly via the
  separable banded-matmul path (Y_r = M_r X M_r^T, two matmul passes).
  The other 15 channels are least-squares linear combinations of the
  basis images (+ the input); fit errors 0.5-3.8e-3 per channel, below
  the bf16 matmul noise (~5e-3) and far below the 2e-2 gate.

  Engine split (measured costs; scalar_tensor_tensor has NO DVE fast
  mode at 2.2us/term, and is not a legal GPSIMD opcode, so combos use
  only tensor_scalar 4x + tensor_tensor 2x on DVE, plus the PE):
   - PE: 3-term targets (and the last channel's late 2-term targets)
     as PSUM accumulations of identity matmuls psum += (c_r*I) @ y_r,
     one 128-col bf16 stationary per coefficient, ~1us/term.
   - DVE: remaining targets via tensor_scalar (4x, 0.71us) per term +
     tensor_tensor add (2x, 1.25us) = ~2.7us per 2-term image.
   - ACT: basis PSUM evacuations + half the PE-combo evacs (DVE takes
     the other half); first items' evacs alternate ACT/DVE to ramp the
     output DMA stream early.
  Scheduling: combos emit one basis item after their last term's
  producer (same-item emission hangs the exec unit - observed thrice,
  incl. the sched clamp putting a late ci=2 PE combo on its producer's
  item); x/w loads are chunked across DMA queues (~24GB/s per queue);
  y writes stay full-width (partition-split DMA measured 24us slower);
  16-deep output pools cover the ~21us per-write DMA latency.
  DMA (33MB f16 out, ~100-110us active at ~360GB/s) is the roofline
  wall: measured 121.9-125.6us vs the 164.8us all-direct baseline.
"""

import numpy as np
import ml_dtypes

B, C, H, W = 8, 3, 512, 512
N = 512
P = 128
NBLK = N // P  # 4

NUM_KERNELS = 21
MAX_KSIZE = 21
INIT_KSIZE = 3
_INCREMENT = (MAX_KSIZE - INIT_KSIZE) / (NUM_KERNELS - 2)
KSIZES = [
    min(MAX_KSIZE, int(INIT_KSIZE + i * _INCREMENT // 2 * 2))
    for i in range(NUM_KERNELS)
]
SIGMAS = np.linspace(0.5, 1.0, NUM_KERNELS)

# Basis kernels computed directly on the PE; everything else is a linear
# combination of basis outputs (and the input image, key -1).
BASIS = [0, 5, 8, 12, 16, 20]
# target kernel -> [(source, coeff)]; source -1 = input x, else basis kernel.
PLAN = {
    1: [(0, 0.653574), (5, 0.632715), (8, -0.287497)],    # 7.2e-04
    2: [(0, 0.382643), (5, 1.043553), (8, -0.426767)],    # 1.0e-03
    3: [(-1, -0.063554), (0, 0.511930), (5, 0.553370)],   # 1.1e-03
    4: [(0, 0.076053), (5, 1.145205), (8, -0.222347)],    # 8.5e-04
    6: [(5, 0.619732), (8, 0.383371)],                    # 2.7e-03
    7: [(5, 0.288176), (8, 0.714818)],                    # 2.7e-03
    9: [(8, 0.702238), (12, 0.301988)],                   # 3.8e-03
    10: [(8, 0.328289), (12, 0.853152), (16, -0.182158)], # 7.5e-04
    11: [(8, 0.205813), (12, 0.798077)],                  # 3.6e-03
    13: [(12, 0.711159), (16, 0.292056)],                 # 3.2e-03
    14: [(12, 0.329416), (16, 0.851055), (20, -0.181071)],# 5.5e-04
    15: [(12, 0.213595), (16, 0.789372)],                 # 3.0e-03
    17: [(16, 0.718522), (20, 0.283811)],                 # 2.4e-03
    18: [(16, 0.459025), (20, 0.543984)],                 # 3.1e-03
    19: [(16, 0.219984), (20, 0.782200)],                 # 2.3e-03
}
PE_TARGETS = [1, 2, 4, 10, 14]   # identity-matmul PSUM accumulation
# For the LAST channel, the late-ready 2-term targets also go to the PE:
# they become ready only after the final basis item, when PE/ACT are
# otherwise drained while DVE still has a tail of work.
PE_TARGETS_LAST = [1, 2, 4, 10, 14, 15, 17, 18]
# Mid-schedule 2-term targets shifted DVE->PE for the first two channels
# (DVE is the most-loaded engine at ~100us, PE ~90).
PE_TARGETS_EARLY = [1, 2, 4, 9, 10, 13, 14]
WID_TARGETS = [1, 2, 4, 9, 10, 13, 14, 15, 17, 18, 19]
# scalar_tensor_tensor is NOT a legal Pool-engine opcode on V3 (walrus
# ISA check rejects it), so GPSIMD takes no combo targets.
GP_TARGETS = []
# everything else in PLAN runs on DVE (ts/ts/tt_add structure)

TRACE = False  # set True (from a driver) to capture an NTFF profile
LAST_RESULTS = {}  # driver-inspectable: exec_time_ns etc.


def _gauss1d(k, sigma):
    x = np.arange(k, dtype=np.float32)
    g = np.exp(-((x - k // 2) ** 2) / (2.0 * sigma**2))
    return g / g.sum()


def _conv_matrix(g, n=N):
    """Banded matrix M (float64) s.t. y = M @ x computes the reflect-padded
    1D convolution with taps g."""
    k = len(g)
    p = (k - 1) // 2
    M = np.zeros((n, n), np.float64)
    for r in range(n):
        for t in range(k):
            c = r + t - p
            if c < 0:
                c = -c
            elif c >= n:
                c = 2 * (n - 1) - c
            M[r, c] += g[t]
    return M


def _slab_geometry():
    """Per (basis kernel, block) slab column ranges in M^T, plus offsets."""
    geo = []  # [bi][b] = (clo, chi, off)
    off = 0
    for r in BASIS:
        p = (KSIZES[r] - 1) // 2
        row = []
        for b in range(NBLK):
            clo = max(0, P * b - p)
            chi = min(N, P * b + P + p)
            row.append((clo, chi, off))
            off += chi - clo
        geo.append(row)
    return geo, off


def _build_wpack():
    geo, totalw = _slab_geometry()
    wpack = np.zeros((P, totalw), ml_dtypes.bfloat16)
    for bi, r in enumerate(BASIS):
        MT = _conv_matrix(_gauss1d(KSIZES[r], SIGMAS[r])).T
        for b in range(NBLK):
            clo, chi, off = geo[bi][b]
            wpack[:, off : off + (chi - clo)] = MT[P * b : P * b + P, clo:chi].astype(
                ml_dtypes.bfloat16
            )
    return geo, totalw, wpack


def _build_wid():
    """Scaled f16 identity stationaries for the PE-path combo targets.
    Returns (wid array [P, 128*n], {(tk, term_idx): col_off})."""
    n = sum(len(PLAN[tk]) for tk in WID_TARGETS)
    wid = np.zeros((P, P * n), ml_dtypes.bfloat16)
    idoff = {}
    col = 0
    eye = np.eye(P, dtype=np.float32)
    for tk in WID_TARGETS:
        for ti, (_, cf) in enumerate(PLAN[tk]):
            wid[:, col : col + P] = (eye * np.float32(cf)).astype(ml_dtypes.bfloat16)
            idoff[(tk, ti)] = col
            col += P
    return wid, idoff


_GEO, _TOTALW, _WPACK = None, None, None
_WID, _IDOFF = None, None
_NC = None


def _consts():
    global _GEO, _TOTALW, _WPACK, _WID, _IDOFF
    if _WPACK is None:
        _GEO, _TOTALW, _WPACK = _build_wpack()
        _WID, _IDOFF = _build_wid()
    return _GEO, _TOTALW, _WPACK, _WID, _IDOFF


def _build_nc():
    import concourse.bacc as bacc
    import concourse.mybir as mybir
    from concourse.tile import TileContext

    geo, totalw, _, wid_np, idoff = _consts()
    bf16 = mybir.dt.bfloat16
    f16 = mybir.dt.float16
    f32 = mybir.dt.float32
    MULT = mybir.AluOpType.mult
    ADD = mybir.AluOpType.add

    nc = bacc.Bacc("TRN2", target_bir_lowering=False)
    x = nc.dram_tensor("x", [C, P, NBLK * N], bf16, kind="ExternalInput")
    w = nc.dram_tensor("w", [P, totalw], bf16, kind="ExternalInput")
    wid = nc.dram_tensor("wid", list(wid_np.shape), bf16, kind="ExternalInput")
    y = nc.dram_tensor("y", [C * NUM_KERNELS, P, NBLK * N], f16, kind="ExternalOutput")

    # PE-combo PSUM evacuation rotation: DVE is the most-loaded engine
    # (~99us vs ACT ~88), so ACT takes 2/3 of the PE-combo evacs.
    evac_cycle = ["d", "a"]
    necopy = 0
    # Basis evacs go to ACT, except for the first items of channel 0 where
    # DVE is still idle (its first combos only become ready at item ~2):
    # alternating halves the per-item evac latency there, which ramps the
    # output DMA stream ~10us sooner.
    early_split_items = 3
    cur_item = [0]
    nbcopy = [0]

    with TileContext(nc) as tc:
        with (
            tc.tile_pool(name="wsb", bufs=1) as wpool,
            tc.tile_pool(name="wid", bufs=1) as widpool,
            tc.tile_pool(name="xsb", bufs=3) as xpool,
            tc.tile_pool(name="zt", bufs=6) as ztpool,
            tc.tile_pool(name="yb", bufs=16) as ybpool,
            tc.tile_pool(name="yc", bufs=16) as ycpool,
            tc.tile_pool(name="ps", bufs=4, space="PSUM") as ps,
        ):
            xch = {}
            t = xpool.tile([P, NBLK * N], bf16, tag="x")
            for j in range(4):
                nc.sync.dma_start(t[:, j * N : (j + 1) * N], x[0][:, j * N : (j + 1) * N])
            xch[0] = t

            wsb = wpool.tile([P, totalw], bf16)
            k0_end = geo[1][0][2]
            bounds = [0, k0_end // 2, k0_end] + [geo[bi][0][2] for bi in (2, 4)] + [totalw]
            for a, b in zip(bounds[:-1], bounds[1:]):
                nc.sync.dma_start(wsb[:, a:b], w[:, a:b])
            widsb = widpool.tile(list(wid_np.shape), bf16)
            nc.sync.dma_start(widsb[:], wid[:])

            for ci in range(1, C):
                t = xpool.tile([P, NBLK * N], bf16, tag="x")
                for j in range(4):
                    nc.sync.dma_start(t[:, j * N : (j + 1) * N], x[ci][:, j * N : (j + 1) * N])
                xch[ci] = t

            ybase = {}  # (ci, basis kernel) -> resident f16 image tile

            def basis_evac(dst, src):
                if cur_item[0] < early_split_items:
                    nbcopy[0] += 1
                    if nbcopy[0] % 2 == 0:
                        nc.vector.tensor_copy(dst, src)
                        return
                nc.scalar.copy(dst, src)

            def emit_pass1(ci, bi):
                # ---- pass 1: Z^T[wb] = sum_j X[j,wb]^T @ slab(bi,j) ----
                xt = xch[ci]
                zt = []
                for wb2 in range(NBLK // 2):
                    psz = ps.tile([P, 2 * N], f32, tag="ps")
                    for half in range(2):
                        wb = 2 * wb2 + half
                        for j in range(NBLK):
                            clo, chi, off = geo[bi][j]
                            nc.tensor.matmul(
                                psz[:, half * N + clo : half * N + chi],
                                xt[:, N * j + P * wb : N * j + P * wb + P],
                                wsb[:, off : off + (chi - clo)],
                                start=(j == 0),
                                stop=(j == NBLK - 1),
                            )
                    zt2 = ztpool.tile([P, 2 * N], bf16, tag="zt")
                    basis_evac(zt2[:], psz[:])
                    zt.append(zt2)
                return zt

            def emit_pass2(ci, bi, zt):
                # ---- pass 2: Y[hb] = sum_wb Z^T[wb,hb]^T @ slab(bi,wb) ----
                def ztap(wb, hb):
                    return zt[wb // 2][:, (wb % 2) * N + P * hb : (wb % 2) * N + P * hb + P]

                r = BASIS[bi]
                cout = C * r + ci
                yo = ybpool.tile([P, 4 * N], f16, tag="yb")
                for hb2 in range(NBLK // 2):
                    psy = ps.tile([P, 2 * N], f32, tag="ps")
                    for half in range(2):
                        hb = 2 * hb2 + half
                        for wb in range(NBLK):
                            clo, chi, off = geo[bi][wb]
                            nc.tensor.matmul(
                                psy[:, half * N + clo : half * N + chi],
                                ztap(wb, hb),
                                wsb[:, off : off + (chi - clo)],
                                start=(wb == 0),
                                stop=(wb == NBLK - 1),
                            )
                    basis_evac(yo[:, hb2 * 2 * N : (hb2 + 1) * 2 * N], psy[:])
                dma_out(cout, yo)
                ybase[(ci, r)] = yo

            def src_tile(ci, s):
                return xch[ci] if s < 0 else ybase[(ci, s)]

            def dma_out(cout, tile):
                # one full-width write: DMA reads all 128 partitions in
                # parallel; partition-split halves measured 24us slower
                nc.sync.dma_start(y[cout], tile[:])

            def emit_pe_combo(ci, tk):
                # psum[half] = sum_r (c_r * I) @ y_r[half]; evac DVE/ACT.
                nonlocal necopy
                terms = PLAN[tk]
                cout = C * tk + ci
                acc = ycpool.tile([P, 4 * N], f16, tag="yc")
                for half in range(2):
                    pst = ps.tile([P, 2 * N], f32, tag="ps")
                    # q outer: keep each PSUM bank's accumulation group
                    # sequential (matmul output must stay within one
                    # 512-col PSUM bank)
                    for q in range(2):
                        for ti, (s, _) in enumerate(terms):
                            off = idoff[(tk, ti)]
                            nc.tensor.matmul(
                                pst[:, q * N : (q + 1) * N],
                                widsb[:, off : off + P],
                                src_tile(ci, s)[
                                    :, half * 2 * N + q * N : half * 2 * N + (q + 1) * N
                                ],
                                start=(ti == 0),
                                stop=(ti == len(terms) - 1),
                            )
                    e = evac_cycle[necopy % 2]
                    necopy += 1
                    if e == "a":
                        nc.scalar.copy(acc[:, half * 2 * N : (half + 1) * 2 * N], pst[:])
                    else:
                        nc.vector.tensor_copy(acc[:, half * 2 * N : (half + 1) * 2 * N], pst[:])
                dma_out(cout, acc)

            def emit_dve_combo(ci, tk):
                # t1 = c0*u ; t2 = c1*v ; t1 += t2 ; [t2 = c2*w ; t1 += t2]
                terms = PLAN[tk]
                cout = C * tk + ci
                t1 = ycpool.tile([P, 4 * N], f16, tag="yc")
                t2 = ycpool.tile([P, 4 * N], f16, tag="yc")
                nc.vector.tensor_scalar_mul(t1[:], src_tile(ci, terms[0][0])[:], float(terms[0][1]))
                nc.vector.tensor_scalar_mul(t2[:], src_tile(ci, terms[1][0])[:], float(terms[1][1]))
                nc.vector.tensor_tensor(t1[:], t1[:], t2[:], ADD)
                for (s, cf) in terms[2:]:
                    nc.vector.tensor_scalar_mul(t2[:], src_tile(ci, s)[:], float(cf))
                    nc.vector.tensor_tensor(t1[:], t1[:], t2[:], ADD)
                dma_out(cout, t1)

            def emit_gp_combo(ci, tk):
                terms = PLAN[tk]
                cout = C * tk + ci
                acc = ycpool.tile([P, 4 * N], f16, tag="yc")
                nc.gpsimd.tensor_scalar_mul(acc[:], src_tile(ci, terms[0][0])[:], float(terms[0][1]))
                for (s, cf) in terms[1:]:
                    nc.gpsimd.scalar_tensor_tensor(
                        acc[:], src_tile(ci, s)[:], float(cf), acc[:], MULT, ADD
                    )
                dma_out(cout, acc)

            # Software-pipeline by one stage (pass 1 of item k+1 before
            # pass 2 of item k).  Each combo is emitted one basis item
            # AFTER its last term image is produced (readiness + one item
            # of slack for the ACT evacuation to land), which spreads
            # DVE/PE combo work and output DMA across the whole timeline
            # instead of clumping it at channel boundaries.
            items = [(ci, bi) for ci in range(C) for bi in range(len(BASIS))]
            b_idx = {r: bi for bi, r in enumerate(BASIS)}
            sched = {}  # global item idx -> [(ci, tk, engine)]
            for ci in range(C):
                pe_set = PE_TARGETS_LAST if ci == C - 1 else PE_TARGETS
                for tk, terms in PLAN.items():
                    ready = max(b_idx[s] for s, _ in terms if s >= 0)
                    eng = "pe" if tk in pe_set else ("gp" if tk in GP_TARGETS else "dve")
                    # +1 item of slack after the last term's producer:
                    # same-item emission hangs the device (exec-unit
                    # unrecoverable), observed twice.
                    at = min(ci * len(BASIS) + ready + 1, len(items) - 1)
                    sched.setdefault(at, []).append((ci, tk, eng))

            def emit_combo(ci, tk, eng):
                if eng == "pe":
                    emit_pe_combo(ci, tk)
                elif eng == "gp":
                    emit_gp_combo(ci, tk)
                else:
                    emit_dve_combo(ci, tk)

            zt_prev = emit_pass1(*items[0])
            for idx, (ci, bi) in enumerate(items):
                cur_item[0] = idx
                zt_next = (
                    emit_pass1(*items[idx + 1]) if idx + 1 < len(items) else None
                )
                emit_pass2(ci, bi, zt_prev)
                for (cci, tk, eng) in sched.get(idx, []):
                    emit_combo(cci, tk, eng)
                zt_prev = zt_next

    nc.finalize()
    return nc


def _get_nc():
    global _NC
    if _NC is None:
        _NC = _build_nc()
    return _NC


def _install_trace_hook():
    """Best-effort NTFF profiling hook for axon (used when TRACE=True)."""
    import sys
    import types

    if "antenv.axon_hooks" in sys.modules:
        return
    m = types.ModuleType("antenv.axon_hooks")
    m._hook = None
    m.set_axon_ntff_profile_hook = lambda h: setattr(m, "_hook", h)
    m.get_axon_ntff_profile_hook = lambda: m._hook
    sys.modules["antenv.axon_hooks"] = m
    try:
        import antenv

        antenv.axon_hooks = m
        from trn_agent_boot.trn_boot import _ntff_profile_via_ctypes

        m._hook = _ntff_profile_via_ctypes("/opt/axon/libaxon_pjrt.so")
    except Exception:
        pass


def kernel(x):
    import concourse.bass_utils as bass_utils

    x = np.asarray(x, dtype=np.float32)
    assert x.shape == (B, C, H, W), x.shape
    _, _, wpack, wid, _ = _consts()
    nc = _get_nc()

    # partition-major device layout: x_perm[c, p, b*512+w] = x[c, 128b+p, w]
    x_bf = np.ascontiguousarray(
        x.astype(ml_dtypes.bfloat16)
        .reshape(B, C, NBLK, P, W)
        .transpose(0, 1, 3, 2, 4)
        .reshape(B, C, P, NBLK * W)
    )
    in_maps = [{"x": x_bf[b], "w": wpack, "wid": wid} for b in range(B)]
    kwargs = {}
    if TRACE:
        _install_trace_hook()
        bass_utils.upload_artifacts = lambda tmpdir: "local://" + tmpdir
        kwargs["trace"] = True
    res = bass_utils.run_bass_kernel_spmd(
        nc, in_maps, core_ids=list(range(B)), **kwargs
    )
    LAST_RESULTS["exec_time_ns"] = res.exec_time_ns
    LAST_RESULTS["mean_exec_time_ns"] = res.mean_exec_time_ns

    out = np.empty((B, C * (NUM_KERNELS + 1), H, W), np.float32)
    out[:, :C] = x
    for b in range(B):
        yb = res.results[b]["y"]  # [63, 128, 2048] f16, partition-major
        out[b, C:] = (
            yb.astype(np.float32)
            .reshape(C * NUM_KERNELS, P, NBLK, W)
            .transpose(0, 2, 1, 3)
            .reshape(C * NUM_KERNELS, H, W)
        )
    return out
